# revision 6
# baseline (speedup 1.0000x reference)
"""Trainium2 Bass kernel for EuclideanCodebook (VQ) encode+decode.

Computes, for x:[8,1024,256] f32 and embed:[8192,256] f32:
    ind[b,t]   = argmin_n ||x[b,t]-embed[n]||^2
    quant[b,t] = embed[ind[b,t]]

Sharding: data-parallel over the 8192 tokens across 8 NeuronCores
(1024 tokens/core); the codebook is replicated.

Per-core device algorithm (token tiles of 128 on partitions):
  - scores s[t,n] = x[t]·e[n] - 0.5||e[n]||^2  (argmax_n s == argmin_n dist)
    computed on the PE: two K=128 matmuls for x·e plus one "bias matmul"
    (lhsT = e0 selector row) that broadcasts -0.5||e||^2 into PSUM.
  - running cummax of s along n via DVE tensor_tensor_scan (PSUM->SBUF);
    the last element is the global max per token.
  - index = count of positions where cummax < max (== first argmax), taken
    with a single fused pass (ScalarE Sign+accum, or DVE is_lt+accum).
  - decode = indirect-DMA gather of embed rows from HBM by index.
"""

import numpy as np

import concourse.bass as bass
import concourse.tile as tile
from concourse import bacc, mybir
from concourse.bass_utils import run_bass_kernel_spmd

B, T, D, N = 8, 1024, 256, 8192
NCORES = 8
TPC = (B * T) // NCORES          # tokens per core
NTILES = TPC // 128              # token tiles per core
NCHUNK = 2048                    # psum chunk width (4 banks)
NP = N // NCHUNK                 # chunks per token tile
F32 = mybir.dt.float32
NEG = -3.0e38

# How the count-pass is computed:
#   "act_sign": ScalarE activation(Sign)+accum  (assumes Sign(0)=0 -> idx=-acc)
#   "act_sign1": same but Sign(0)=+1 convention -> idx=(N-acc)/2
#   "dve": DVE tensor_scalar(is_lt)+accum       (idx=acc)
COUNT_MODE = "act_sign"


def _build_nc(count_mode=COUNT_MODE, reps=1):
    nc = bacc.Bacc(
        "TRN2",
        target_bir_lowering=False,
        debug=False,
        num_devices=NCORES,
    )
    xT_d = nc.dram_tensor("xT", [2, 128, TPC], F32, kind="ExternalInput")
    eT_d = nc.dram_tensor("embedT", [2, 128, N], F32, kind="ExternalInput")
    emb_d = nc.dram_tensor("embed", [N, D], F32, kind="ExternalInput")
    nh_d = nc.dram_tensor("nhesq", [1, N], F32, kind="ExternalInput")
    q_d = nc.dram_tensor("quant", [TPC, D], F32, kind="ExternalOutput")
    i_d = nc.dram_tensor("ind", [TPC], mybir.dt.int32, kind="ExternalOutput")

    with tile.TileContext(nc) as tc:
        with (
            tc.tile_pool(name="const", bufs=1) as cpool,
            tc.tile_pool(name="psum", bufs=2, space="PSUM") as ppool,
            tc.tile_pool(name="scores", bufs=1) as spool,
            tc.tile_pool(name="small", bufs=2) as mpool,
            tc.tile_pool(name="gath", bufs=2) as gpool,
        ):
            # ---- resident inputs -------------------------------------------------
            embT = cpool.tile([128, 2, N], F32)
            for c in range(2):
                nc.sync.dma_start(embT[:, c, :], eT_d[c])
            xT = cpool.tile([128, 2, TPC], F32)
            for c in range(2):
                nc.sync.dma_start(xT[:, c, :], xT_d[c])
            # bias tile: row 0 = -0.5*||e||^2, rows 1..127 zeroed (selector
            # matmul multiplies them by 0, but NaN garbage would poison 0*x)
            biasr = cpool.tile([128, N], F32)
            nc.gpsimd.memset(biasr[:], 0.0)
            nc.sync.dma_start(biasr[0:1, :], nh_d[:])
            # e0 selector: out[m,n] = sum_k sel[k,m]*rhs[k,n] = rhs[0,n]
            sel = cpool.tile([128, 128], F32)
            nc.gpsimd.memset(sel[:], 0.0)
            nc.gpsimd.memset(sel[0:1, :], 1.0)
            neginf = cpool.tile([128, 1], F32)
            nc.gpsimd.memset(neginf[:], NEG)
            idxstage = cpool.tile([128, NTILES], mybir.dt.int32)

            for _rep in range(reps):
              for t in range(NTILES):
                s_t = spool.tile([128, N], F32, tag="scores")
                for p in range(NP):
                    ps = ppool.tile([128, NCHUNK], F32, tag="ps")
                    nsl = [
                        slice((p * NCHUNK + s * 512), (p * NCHUNK + (s + 1) * 512))
                        for s in range(NCHUNK // 512)
                    ]
                    for s, sl in enumerate(nsl):
                        nc.tensor.matmul(
                            ps[:, s * 512 : (s + 1) * 512],
                            lhsT=sel[:],
                            rhs=biasr[:, sl],
                            start=True,
                            stop=False,
                        )
                    for c in range(2):
                        for s, sl in enumerate(nsl):
                            nc.tensor.matmul(
                                ps[:, s * 512 : (s + 1) * 512],
                                lhsT=xT[:, c, t * 128 : (t + 1) * 128],
                                rhs=embT[:, c, sl],
                                start=False,
                                stop=(c == 1),
                            )
                    # cummax chunk (chained through previous chunk's last col)
                    init = NEG if p == 0 else s_t[:, p * NCHUNK - 1 : p * NCHUNK]
                    nc.vector.tensor_tensor_scan(
                        out=s_t[:, p * NCHUNK : (p + 1) * NCHUNK],
                        data0=ps[:],
                        data1=neginf[:].to_broadcast([128, NCHUNK]),
                        initial=init,
                        op0=mybir.AluOpType.max,
                        op1=mybir.AluOpType.max,
                    )
                gmax = s_t[:, N - 1 : N]
                acc = mpool.tile([128, 1], F32, tag="acc")
                idxf = mpool.tile([128, 1], F32, tag="idxf")
                if count_mode in ("act_sign", "act_sign1"):
                    ngmax = mpool.tile([128, 1], F32, tag="ngmax")
                    nc.vector.tensor_scalar_mul(ngmax[:], gmax, -1.0)
                    junk = spool.tile([128, N], mybir.dt.bfloat16, tag="junk")
                    nc.scalar.activation(
                        out=junk[:],
                        in_=s_t[:],
                        func=mybir.ActivationFunctionType.Sign,
                        bias=ngmax[:],
                        scale=1.0,
                        accum_out=acc[:],
                    )
                    if count_mode == "act_sign":
                        # cummax<gmax -> -1 (idx terms), ==gmax -> 0: acc = -idx
                        nc.vector.tensor_scalar_mul(idxf[:], acc[:], -1.0)
                    else:
                        # Sign(0)=+1: acc = (N-idx) - idx -> idx = (N-acc)/2
                        nc.vector.tensor_scalar(
                            idxf[:],
                            acc[:],
                            -0.5,
                            float(N) / 2.0,
                            mybir.AluOpType.mult,
                            mybir.AluOpType.add,
                        )
                else:
                    gm = mpool.tile([128, 1], F32, tag="gm")
                    nc.vector.tensor_copy(gm[:], gmax)
                    junk = spool.tile([128, N], F32, tag="junkf")
                    nc.vector.tensor_scalar(
                        junk[:],
                        s_t[:],
                        gm[:],
                        None,
                        mybir.AluOpType.is_lt,
                        accum_out=idxf[:],
                    )
                nc.vector.tensor_copy(idxstage[:, t : t + 1], idxf[:])
                g = gpool.tile([128, D], F32, tag="g")
                nc.gpsimd.indirect_dma_start(
                    out=g[:],
                    out_offset=None,
                    in_=emb_d[:],
                    in_offset=bass.IndirectOffsetOnAxis(
                        ap=idxstage[:, t : t + 1], axis=0
                    ),
                )
                nc.sync.dma_start(q_d[t * 128 : (t + 1) * 128, :], g[:])
            nc.sync.dma_start(i_d.rearrange("(t p) -> p t", p=128), idxstage[:])
    nc.compile()
    return nc


def _prep_inputs(x, embed):
    x = np.ascontiguousarray(np.asarray(x), dtype=np.float32)
    embed = np.ascontiguousarray(np.asarray(embed), dtype=np.float32)
    xf = x.reshape(B * T, D)
    nhesq = (-0.5 * np.sum(embed * embed, axis=1, dtype=np.float32)).astype(
        np.float32
    ).reshape(1, N)
    embedT = np.ascontiguousarray(embed.T).reshape(2, 128, N)
    in_maps = []
    for c in range(NCORES):
        xs = xf[c * TPC : (c + 1) * TPC]
        xTc = np.ascontiguousarray(xs.T).reshape(2, 128, TPC)
        in_maps.append(
            {"xT": xTc, "embedT": embedT, "embed": embed, "nhesq": nhesq}
        )
    return in_maps


def _postprocess(results):
    quant = np.concatenate([r["quant"] for r in results], axis=0)
    ind = np.concatenate([r["ind"] for r in results], axis=0)
    return (
        quant.reshape(B, T, D).astype(np.float32),
        ind.reshape(B, T).astype(np.int32),
    )


def run(x, embed, count_mode=COUNT_MODE, **run_kwargs):
    in_maps = _prep_inputs(x, embed)
    nc = _build_nc(count_mode)
    res = run_bass_kernel_spmd(nc, in_maps, core_ids=list(range(NCORES)), **run_kwargs)
    return _postprocess(res.results), res


def kernel(x, embed):
    (quant, ind), _ = run(x, embed)
    return quant, ind


# revision 30
# speedup vs baseline: 6.2329x; 6.2329x over previous
"""Trainium2 Bass kernel for EuclideanCodebook (VQ) encode+decode.

Computes, for x:[8,1024,256] f32 and embed:[8192,256] f32:
    ind[b,t]   = argmin_n ||x[b,t]-embed[n]||^2
    quant[b,t] = embed[ind[b,t]]

Sharding: data-parallel over the 8192 tokens across 8 NeuronCores
(1024 tokens/core); the codebook is replicated.

Per-core device algorithm (token tiles of 128 on partitions):
  - scores s[t,n] = x[t]·e[n] - 0.5||e[n]||^2  (argmax_n s == argmin_n dist)
    computed on the PE: two K=128 matmuls for x·e plus one "bias matmul"
    (lhsT = e0 selector row) that broadcasts -0.5||e||^2 into PSUM.
  - running cummax of s along n via DVE tensor_tensor_scan (PSUM->SBUF);
    the last element is the global max per token.
  - index = count of positions where cummax < max (== first argmax), taken
    with a single fused pass (ScalarE Sign+accum, or DVE is_lt+accum).
  - decode = indirect-DMA gather of embed rows from HBM by index.
"""

import numpy as np

import concourse.bass as bass
import concourse.tile as tile
from concourse import bacc, mybir
from concourse.bass_utils import run_bass_kernel_spmd

B, T, D, N = 8, 1024, 256, 8192
NCORES = 8
TPC = (B * T) // NCORES          # tokens per core
NTILES = TPC // 128              # token tiles per core
NCHUNK = 2048                    # psum chunk width (4 banks)
NP = N // NCHUNK                 # chunks per token tile
F32 = mybir.dt.float32
F16 = mybir.dt.float16
NEG = -3.0e38

# The PE runs fp32 matmuls at 1/4 rate (2 half-speed passes). Instead we
# split each fp32 operand into a fp16 hi+lo pair (x = hx+lx exactly to
# ~2^-23 rel) and compute hx*he + lx*he + hx*le in full-rate fp16 matmuls;
# fp16 11-bit products are exact in the fp32 PSUM accumulate, and the
# dropped lx*le term is ~1e-6 — same accuracy class as the fp32 matmul.

# How the count-pass is computed:
#   "act_sign": ScalarE activation(Sign)+accum  (assumes Sign(0)=0 -> idx=-acc)
#   "act_sign1": same but Sign(0)=+1 convention -> idx=(N-acc)/2
#   "dve": DVE tensor_scalar(is_lt)+accum       (idx=acc)
COUNT_MODE = "act_sign"

# Scoring/argmax strategy:
#   "scan": bias folded into PSUM via selector matmul; DVE cummax scan;
#           index = count(cummax < max) fused on ScalarE (or DVE)
#   "add_scan": bias added by DVE tensor_tensor add (no bias matmuls),
#           then full-field DVE cummax scan; same count pass
#   "ttr":  (BROKEN on this HW: InstTensorTensorReduce faults the device)
SCORE_MODE = "scan"


def _build_nc(count_mode=COUNT_MODE, reps=1, score_mode=SCORE_MODE):
    nc = bacc.Bacc(
        "TRN2",
        target_bir_lowering=False,
        debug=False,
        num_devices=NCORES,
    )
    # hi/lo fp16 pairs, pre-split on host: index 0,1 = hx chunks, 2,3 = lx
    xT_d = nc.dram_tensor("xT", [4, 128, TPC], F16, kind="ExternalInput")
    eT_d = nc.dram_tensor("embedT", [4, 128, N], F16, kind="ExternalInput")
    emb_d = nc.dram_tensor("embed", [N, D], F32, kind="ExternalInput")
    # rows 0/1 = hi/lo fp16 of -0.5*||e||^2
    nh_d = nc.dram_tensor("nhesq", [2, N], F16, kind="ExternalInput")
    q_d = nc.dram_tensor("quant", [TPC, D], F32, kind="ExternalOutput")
    i_d = nc.dram_tensor("ind", [TPC], mybir.dt.int32, kind="ExternalOutput")

    with tile.TileContext(nc) as tc:
        with (
            tc.tile_pool(name="const", bufs=1) as cpool,
            tc.tile_pool(name="psum", bufs=2, space="PSUM") as ppool,
            tc.tile_pool(name="scores", bufs=1) as spool,
            tc.tile_pool(name="small", bufs=2) as mpool,
            tc.tile_pool(name="gath", bufs=2) as gpool,
        ):
            # ---- resident inputs -------------------------------------------------
            # small control tensors ride the SWDGE queue so they don't sit
            # behind the 8MB embT stream on the HWDGE ring
            xT = cpool.tile([128, 4, TPC], F16)
            for c in range(4):
                nc.gpsimd.dma_start(xT[:, c, :], xT_d[c])
            # embT arrives column-slice-major so the first PSUM chunk's
            # matmuls only wait for the first ~2MB
            embT = cpool.tile([128, 4, N], F16)
            for p in range(NP):
                sl = slice(p * NCHUNK, (p + 1) * NCHUNK)
                for c in range(4):
                    nc.sync.dma_start(embT[:, c, sl], eT_d[c, :, sl])
            # bias tile: rows 0/1 = hi/lo of -0.5*||e||^2, rest zeroed
            # (selector matmul multiplies them by 0, but NaN garbage would
            # poison 0*x). In add_scan mode it is only read during the nhb
            # broadcast prologue, so it shares a slot with the throwaway
            # activation output.
            if score_mode == "add_scan":
                biasr = spool.tile([128, N], F16, tag="bjunk")
            else:
                biasr = cpool.tile([128, N], F16)
            nc.vector.memset(biasr[:], 0.0)
            nc.gpsimd.dma_start(biasr[0:2, :], nh_d[:])
            # row0+row1 selector: out[m,n] = rhs[0,n] + rhs[1,n]
            sel = cpool.tile([128, 128], F16)
            nc.vector.memset(sel[:], 0.0)
            nc.vector.memset(sel[0:2, :], 1.0)
            neginf = cpool.tile([128, 1], F32)
            nc.vector.memset(neginf[:], NEG)
            idxstage = cpool.tile([128, NTILES], mybir.dt.int32)

            if score_mode in ("ttr", "add_scan"):
                # broadcast of -0.5||e||^2 to all partitions, fp32, via the
                # selector matmul (row0+row1 of biasr) and ScalarE copy-back
                nhb = cpool.tile([128, N], F32)
                for p in range(NP):
                    bps = ppool.tile([128, NCHUNK], F32, tag="ps")
                    for s in range(NCHUNK // 512):
                        sl = slice(p * NCHUNK + s * 512, p * NCHUNK + (s + 1) * 512)
                        nc.tensor.matmul(
                            bps[:, s * 512 : (s + 1) * 512],
                            lhsT=sel[:],
                            rhs=biasr[:, sl],
                            start=True,
                            stop=True,
                        )
                    nc.scalar.activation(
                        out=nhb[:, p * NCHUNK : (p + 1) * NCHUNK],
                        in_=bps[:],
                        func=mybir.ActivationFunctionType.Copy,
                        scale=1.0,
                    )

            for _rep in range(reps):
              for t in range(NTILES):
                s_t = spool.tile([128, N], F32, tag="scores")
                if score_mode == "add_scan":
                    cum = spool.tile([128, N], F32, tag="cum")
                if score_mode == "ttr":
                    bmax = mpool.tile([128, 8], F32, tag="bmax")
                    nc.vector.memset(bmax[:], NEG)
                for p in range(NP):
                    ps = ppool.tile([128, NCHUNK], F32, tag="ps")
                    nsl = [
                        slice((p * NCHUNK + s * 512), (p * NCHUNK + (s + 1) * 512))
                        for s in range(NCHUNK // 512)
                    ]
                    if score_mode == "scan":
                        for s, sl in enumerate(nsl):
                            nc.tensor.matmul(
                                ps[:, s * 512 : (s + 1) * 512],
                                lhsT=sel[:],
                                rhs=biasr[:, sl],
                                start=True,
                                stop=False,
                            )
                    # (hx,he), (lx,he), (hx,le) term chunks
                    pairs = [(0, 0), (1, 1), (2, 0), (3, 1), (0, 2), (1, 3)]
                    for pi, (xc, ec) in enumerate(pairs):
                        for s, sl in enumerate(nsl):
                            nc.tensor.matmul(
                                ps[:, s * 512 : (s + 1) * 512],
                                lhsT=xT[:, xc, t * 128 : (t + 1) * 128],
                                rhs=embT[:, ec, sl],
                                start=(score_mode != "scan" and pi == 0),
                                stop=(pi == len(pairs) - 1),
                            )
                    sl_out = slice(p * NCHUNK, (p + 1) * NCHUNK)
                    if score_mode == "scan":
                        # cummax chunk (chained through prev chunk's last col)
                        init = (
                            NEG if p == 0
                            else s_t[:, p * NCHUNK - 1 : p * NCHUNK]
                        )
                        nc.vector.tensor_tensor_scan(
                            out=s_t[:, sl_out],
                            data0=ps[:],
                            data1=neginf[:].to_broadcast([128, NCHUNK]),
                            initial=init,
                            op0=mybir.AluOpType.max,
                            op1=mybir.AluOpType.max,
                        )
                    elif score_mode == "add_scan":
                        nc.vector.tensor_tensor(
                            out=s_t[:, sl_out],
                            in0=ps[:],
                            in1=nhb[:, sl_out],
                            op=mybir.AluOpType.add,
                        )
                        # chunk cummax chained through prev chunk's last col;
                        # interleaved with the adds so PSUM slots free at a
                        # steady cadence and the PE never stalls into HAM
                        # re-throttle
                        init = (
                            NEG if p == 0
                            else cum[:, p * NCHUNK - 1 : p * NCHUNK]
                        )
                        nc.vector.tensor_tensor_scan(
                            out=cum[:, sl_out],
                            data0=s_t[:, sl_out],
                            data1=neginf[:].to_broadcast([128, NCHUNK]),
                            initial=init,
                            op0=mybir.AluOpType.max,
                            op1=mybir.AluOpType.max,
                        )
                    else:
                        # s = xe + (-0.5||e||^2); running chunk max in accum
                        nc.vector.tensor_tensor_reduce(
                            out=s_t[:, sl_out],
                            in0=ps[:],
                            in1=nhb[:, sl_out],
                            scale=1.0,
                            scalar=NEG,
                            op0=mybir.AluOpType.add,
                            op1=mybir.AluOpType.max,
                            accum_out=bmax[:, p : p + 1],
                        )
                if score_mode == "ttr":
                    gmax8 = mpool.tile([128, 8], F32, tag="gmax8")
                    nc.vector.max(gmax8[:], bmax[:])
                    idx8 = mpool.tile([128, 8], mybir.dt.uint32, tag="idx8")
                    nc.vector.max_index(idx8[:], gmax8[:], s_t[:])
                    nc.vector.tensor_copy(
                        idxstage[:, t : t + 1], idx8[:, 0:1]
                    )
                else:
                    if score_mode == "add_scan":
                        field = cum
                        junkb = spool.tile([128, N], F16, tag="bjunk")
                        junk_ap = junkb[:]
                    else:
                        field = s_t
                        junk_ap = None
                    idxf = mpool.tile([128, 1], F32, tag="idxf")
                    gmax = field[:, N - 1 : N]
                    acc = mpool.tile([128, 1], F32, tag="acc")
                    if count_mode in ("act_sign", "act_sign1"):
                        ngmax = mpool.tile([128, 1], F32, tag="ngmax")
                        nc.vector.tensor_scalar_mul(ngmax[:], gmax, -1.0)
                        if junk_ap is None:
                            junk = spool.tile(
                                [128, N], mybir.dt.bfloat16, tag="junk"
                            )
                            junk_ap = junk[:]
                        nc.scalar.activation(
                            out=junk_ap,
                            in_=field[:],
                            func=mybir.ActivationFunctionType.Sign,
                            bias=ngmax[:],
                            scale=1.0,
                            accum_out=acc[:],
                        )
                        if count_mode == "act_sign":
                            # cummax<gmax -> -1; ==gmax -> 0: acc = -idx
                            nc.vector.tensor_scalar_mul(idxf[:], acc[:], -1.0)
                        else:
                            # Sign(0)=+1: acc = (N-idx)-idx -> idx=(N-acc)/2
                            nc.vector.tensor_scalar(
                                idxf[:],
                                acc[:],
                                -0.5,
                                float(N) / 2.0,
                                mybir.AluOpType.mult,
                                mybir.AluOpType.add,
                            )
                    else:
                        gm = mpool.tile([128, 1], F32, tag="gm")
                        nc.vector.tensor_copy(gm[:], gmax)
                        junkf = spool.tile([128, N], F32, tag="junkf")
                        nc.vector.tensor_scalar(
                            junkf[:],
                            field[:],
                            gm[:],
                            None,
                            mybir.AluOpType.is_lt,
                            accum_out=idxf[:],
                        )
                    nc.vector.tensor_copy(idxstage[:, t : t + 1], idxf[:])
                g = gpool.tile([128, D], F32, tag="g")
                nc.gpsimd.indirect_dma_start(
                    out=g[:],
                    out_offset=None,
                    in_=emb_d[:],
                    in_offset=bass.IndirectOffsetOnAxis(
                        ap=idxstage[:, t : t + 1], axis=0
                    ),
                )
                nc.sync.dma_start(q_d[t * 128 : (t + 1) * 128, :], g[:])
            nc.sync.dma_start(i_d.rearrange("(t p) -> p t", p=128), idxstage[:])
    nc.compile()
    return nc


def _hilo16(a32):
    """Split fp32 array into (hi, lo) fp16 pair with hi+lo ~= a to ~2^-23."""
    hi = a32.astype(np.float16)
    lo = (a32 - hi.astype(np.float32)).astype(np.float16)
    return hi, lo


def _prep_inputs(x, embed):
    x = np.ascontiguousarray(np.asarray(x), dtype=np.float32)
    embed = np.ascontiguousarray(np.asarray(embed), dtype=np.float32)
    xf = x.reshape(B * T, D)
    nh = -0.5 * np.sum(embed * embed, axis=1, dtype=np.float32)
    nh_hi, nh_lo = _hilo16(nh.astype(np.float32))
    nhesq = np.stack([nh_hi, nh_lo]).reshape(2, N)
    eT = np.ascontiguousarray(embed.T)            # [256, 8192] f32
    e_hi, e_lo = _hilo16(eT)
    embedT = np.concatenate(
        [e_hi.reshape(2, 128, N), e_lo.reshape(2, 128, N)], axis=0
    )                                              # [4,128,N] f16
    in_maps = []
    for c in range(NCORES):
        xs = np.ascontiguousarray(xf[c * TPC : (c + 1) * TPC].T)  # [256,TPC]
        x_hi, x_lo = _hilo16(xs)
        xTc = np.concatenate(
            [x_hi.reshape(2, 128, TPC), x_lo.reshape(2, 128, TPC)], axis=0
        )                                          # [4,128,TPC] f16
        in_maps.append(
            {"xT": xTc, "embedT": embedT, "embed": embed, "nhesq": nhesq}
        )
    return in_maps


def _postprocess(results):
    quant = np.concatenate([r["quant"] for r in results], axis=0)
    ind = np.concatenate([r["ind"] for r in results], axis=0)
    return (
        quant.reshape(B, T, D).astype(np.float32),
        ind.reshape(B, T).astype(np.int32),
    )


def run(x, embed, count_mode=COUNT_MODE, score_mode=SCORE_MODE, **run_kwargs):
    in_maps = _prep_inputs(x, embed)
    nc = _build_nc(count_mode, score_mode=score_mode)
    res = run_bass_kernel_spmd(nc, in_maps, core_ids=list(range(NCORES)), **run_kwargs)
    return _postprocess(res.results), res


def kernel(x, embed):
    (quant, ind), _ = run(x, embed)
    return quant, ind


# revision 40
# speedup vs baseline: 6.5567x; 1.0520x over previous
"""Trainium2 Bass kernel for EuclideanCodebook (VQ) encode+decode.

Computes, for x:[8,1024,256] f32 and embed:[8192,256] f32:
    ind[b,t]   = argmin_n ||x[b,t]-embed[n]||^2
    quant[b,t] = embed[ind[b,t]]

Sharding: data-parallel over the 8192 tokens across 8 NeuronCores
(1024 tokens/core); the codebook is replicated.

Per-core device algorithm (token tiles of 128 on partitions):
  - scores s[t,n] = x[t]·e[n] - 0.5||e[n]||^2  (argmax_n s == argmin_n dist)
    computed on the PE: two K=128 matmuls for x·e plus one "bias matmul"
    (lhsT = e0 selector row) that broadcasts -0.5||e||^2 into PSUM.
  - running cummax of s along n via DVE tensor_tensor_scan (PSUM->SBUF);
    the last element is the global max per token.
  - index = count of positions where cummax < max (== first argmax), taken
    with a single fused pass (ScalarE Sign+accum, or DVE is_lt+accum).
  - decode = indirect-DMA gather of embed rows from HBM by index.
"""

import numpy as np

import concourse.bass as bass
import concourse.tile as tile
from concourse import bacc, mybir
from concourse.bass_utils import run_bass_kernel_spmd

B, T, D, N = 8, 1024, 256, 8192
NCORES = 8
TPC = (B * T) // NCORES          # tokens per core
NTILES = TPC // 128              # token tiles per core
NCHUNK = 2048                    # psum chunk width (4 banks)
NP = N // NCHUNK                 # chunks per token tile
F32 = mybir.dt.float32
F16 = mybir.dt.float16
NEG = -3.0e38

# The PE runs fp32 matmuls at 1/4 rate (2 half-speed passes). Instead we
# split each fp32 operand into a fp16 hi+lo pair (x = hx+lx exactly to
# ~2^-23 rel) and compute hx*he + lx*he + hx*le in full-rate fp16 matmuls;
# fp16 11-bit products are exact in the fp32 PSUM accumulate, and the
# dropped lx*le term is ~1e-6 — same accuracy class as the fp32 matmul.

# How the count-pass is computed:
#   "act_sign": ScalarE activation(Sign)+accum  (assumes Sign(0)=0 -> idx=-acc)
#   "act_sign1": same but Sign(0)=+1 convention -> idx=(N-acc)/2
#   "dve": DVE tensor_scalar(is_lt)+accum       (idx=acc)
COUNT_MODE = "act_sign"

# Scoring/argmax strategy:
#   "scan": bias folded into PSUM via selector matmul; DVE cummax scan;
#           index = count(cummax < max) fused on ScalarE (or DVE)
#   "add_scan": bias added by DVE tensor_tensor add (no bias matmuls),
#           then full-field DVE cummax scan; same count pass
#   "ttr":  (BROKEN on this HW: InstTensorTensorReduce faults the device)
SCORE_MODE = "scan"


def _build_nc(count_mode=COUNT_MODE, reps=1, score_mode=SCORE_MODE):
    nc = bacc.Bacc(
        "TRN2",
        target_bir_lowering=False,
        debug=False,
        num_devices=NCORES,
    )
    # hi/lo fp16 pairs, pre-split on host: index 0,1 = hx chunks, 2,3 = lx
    xT_d = nc.dram_tensor("xT", [4, 128, TPC], F16, kind="ExternalInput")
    eT_d = nc.dram_tensor("embedT", [4, 128, N], F16, kind="ExternalInput")
    emb_d = nc.dram_tensor("embed", [N, D], F32, kind="ExternalInput")
    # rows 0/1 = hi/lo fp16 of -0.5*||e||^2
    nh_d = nc.dram_tensor("nhesq", [2, N], F16, kind="ExternalInput")
    # same bias as a single fp32 row (DMA-broadcast source for add_scan)
    nh32_d = nc.dram_tensor("nhesq32", [1, N], F32, kind="ExternalInput")
    q_d = nc.dram_tensor("quant", [TPC, D], F32, kind="ExternalOutput")
    i_d = nc.dram_tensor("ind", [TPC], mybir.dt.int32, kind="ExternalOutput")

    with tile.TileContext(nc) as tc:
        with (
            tc.tile_pool(name="const", bufs=1) as cpool,
            tc.tile_pool(name="psum", bufs=2, space="PSUM") as ppool,
            tc.tile_pool(name="scores", bufs=1) as spool,
            tc.tile_pool(name="small", bufs=2) as mpool,
            tc.tile_pool(name="gath", bufs=2) as gpool,
        ):
            # ---- resident inputs -------------------------------------------------
            # Small control tensors are issued FIRST: DMA completion sems are
            # FIFO per lane, so anything queued after the 8MB embT stream
            # would wait behind it.
            if score_mode != "add_scan":
                # bias tile: rows 0/1 = hi/lo of -0.5*||e||^2, rest zeroed
                # (selector matmul multiplies them by 0, but NaN garbage
                # would poison 0*x)
                biasr = cpool.tile([128, N], F16)
                nc.vector.memset(biasr[:], 0.0)
                nc.gpsimd.dma_start(biasr[0:2, :], nh_d[:])
                # row0+row1 selector: out[m,n] = rhs[0,n] + rhs[1,n]
                sel = cpool.tile([128, 128], F16)
                nc.vector.memset(sel[:], 0.0)
                nc.vector.memset(sel[0:2, :], 1.0)
            neginf = cpool.tile([128, 1], F32)
            nc.vector.memset(neginf[:], NEG)
            xT = cpool.tile([128, 4, TPC], F16)
            for c in range(4):
                nc.gpsimd.dma_start(xT[:, c, :], xT_d[c])
            # embT arrives column-slice-major, interleaved with the bias
            # broadcast: per column slice, hi chunks (consumed by the first
            # four matmul pairs) first, then the DMA-broadcast fp32 bias row
            # (consumed by the DVE add right after the chunk's matmuls),
            # then the lo chunks.
            if score_mode == "add_scan":
                nhb = cpool.tile([128, N], F32)
            embT = cpool.tile([128, 4, N], F16)
            for p in range(NP):
                sl = slice(p * NCHUNK, (p + 1) * NCHUNK)
                for c in (0, 1):
                    nc.sync.dma_start(embT[:, c, sl], eT_d[c, :, sl])
                if score_mode == "add_scan":
                    nc.sync.dma_start(
                        nhb[:, sl],
                        nh32_d[0:1, sl].to_broadcast([128, NCHUNK]),
                    )
                for c in (2, 3):
                    nc.sync.dma_start(embT[:, c, sl], eT_d[c, :, sl])
            idxstage = cpool.tile([128, NTILES], mybir.dt.int32)

            if score_mode == "ttr":
                nhb = cpool.tile([128, N], F32)
                for p in range(NP):
                    bps = ppool.tile([128, NCHUNK], F32, tag="ps")
                    for s in range(NCHUNK // 512):
                        sl = slice(p * NCHUNK + s * 512, p * NCHUNK + (s + 1) * 512)
                        nc.tensor.matmul(
                            bps[:, s * 512 : (s + 1) * 512],
                            lhsT=sel[:],
                            rhs=biasr[:, sl],
                            start=True,
                            stop=True,
                        )
                    nc.scalar.activation(
                        out=nhb[:, p * NCHUNK : (p + 1) * NCHUNK],
                        in_=bps[:],
                        func=mybir.ActivationFunctionType.Copy,
                        scale=1.0,
                    )

            for _rep in range(reps):
              for t in range(NTILES):
                s_t = spool.tile([128, N], F32, tag="scores")
                if score_mode == "add_scan":
                    cum = spool.tile([128, N], F32, tag="cum")
                if score_mode == "ttr":
                    bmax = mpool.tile([128, 8], F32, tag="bmax")
                    nc.vector.memset(bmax[:], NEG)
                for p in range(NP):
                    ps = ppool.tile([128, NCHUNK], F32, tag="ps")
                    nsl = [
                        slice((p * NCHUNK + s * 512), (p * NCHUNK + (s + 1) * 512))
                        for s in range(NCHUNK // 512)
                    ]
                    if score_mode == "scan":
                        for s, sl in enumerate(nsl):
                            nc.tensor.matmul(
                                ps[:, s * 512 : (s + 1) * 512],
                                lhsT=sel[:],
                                rhs=biasr[:, sl],
                                start=True,
                                stop=False,
                            )
                    # (hx,he), (lx,he), (hx,le) term chunks
                    pairs = [(0, 0), (1, 1), (2, 0), (3, 1), (0, 2), (1, 3)]
                    for pi, (xc, ec) in enumerate(pairs):
                        for s, sl in enumerate(nsl):
                            nc.tensor.matmul(
                                ps[:, s * 512 : (s + 1) * 512],
                                lhsT=xT[:, xc, t * 128 : (t + 1) * 128],
                                rhs=embT[:, ec, sl],
                                start=(score_mode != "scan" and pi == 0),
                                stop=(pi == len(pairs) - 1),
                            )
                    sl_out = slice(p * NCHUNK, (p + 1) * NCHUNK)
                    if score_mode == "scan":
                        # cummax chunk (chained through prev chunk's last col)
                        init = (
                            NEG if p == 0
                            else s_t[:, p * NCHUNK - 1 : p * NCHUNK]
                        )
                        nc.vector.tensor_tensor_scan(
                            out=s_t[:, sl_out],
                            data0=ps[:],
                            data1=neginf[:].to_broadcast([128, NCHUNK]),
                            initial=init,
                            op0=mybir.AluOpType.max,
                            op1=mybir.AluOpType.max,
                        )
                    elif score_mode == "add_scan":
                        nc.vector.tensor_tensor(
                            out=s_t[:, sl_out],
                            in0=ps[:],
                            in1=nhb[:, sl_out],
                            op=mybir.AluOpType.add,
                        )
                        # chunk cummax chained through prev chunk's last col;
                        # interleaved with the adds so PSUM slots free at a
                        # steady cadence and the PE never stalls into HAM
                        # re-throttle
                        init = (
                            NEG if p == 0
                            else cum[:, p * NCHUNK - 1 : p * NCHUNK]
                        )
                        nc.vector.tensor_tensor_scan(
                            out=cum[:, sl_out],
                            data0=s_t[:, sl_out],
                            data1=neginf[:].to_broadcast([128, NCHUNK]),
                            initial=init,
                            op0=mybir.AluOpType.max,
                            op1=mybir.AluOpType.max,
                        )
                    else:
                        # s = xe + (-0.5||e||^2); running chunk max in accum
                        nc.vector.tensor_tensor_reduce(
                            out=s_t[:, sl_out],
                            in0=ps[:],
                            in1=nhb[:, sl_out],
                            scale=1.0,
                            scalar=NEG,
                            op0=mybir.AluOpType.add,
                            op1=mybir.AluOpType.max,
                            accum_out=bmax[:, p : p + 1],
                        )
                if score_mode == "ttr":
                    gmax8 = mpool.tile([128, 8], F32, tag="gmax8")
                    nc.vector.max(gmax8[:], bmax[:])
                    idx8 = mpool.tile([128, 8], mybir.dt.uint32, tag="idx8")
                    nc.vector.max_index(idx8[:], gmax8[:], s_t[:])
                    nc.vector.tensor_copy(
                        idxstage[:, t : t + 1], idx8[:, 0:1]
                    )
                else:
                    if score_mode == "add_scan":
                        field = cum
                        junkb = spool.tile([128, N], F16, tag="bjunk")
                        junk_ap = junkb[:]
                    else:
                        field = s_t
                        junk_ap = None
                    idxf = mpool.tile([128, 1], F32, tag="idxf")
                    gmax = field[:, N - 1 : N]
                    acc = mpool.tile([128, 1], F32, tag="acc")
                    if count_mode in ("act_sign", "act_sign1"):
                        ngmax = mpool.tile([128, 1], F32, tag="ngmax")
                        nc.vector.tensor_scalar_mul(ngmax[:], gmax, -1.0)
                        if junk_ap is None:
                            junk = spool.tile(
                                [128, N], mybir.dt.bfloat16, tag="junk"
                            )
                            junk_ap = junk[:]
                        nc.scalar.activation(
                            out=junk_ap,
                            in_=field[:],
                            func=mybir.ActivationFunctionType.Sign,
                            bias=ngmax[:],
                            scale=1.0,
                            accum_out=acc[:],
                        )
                        if count_mode == "act_sign":
                            # cummax<gmax -> -1; ==gmax -> 0: acc = -idx
                            nc.vector.tensor_scalar_mul(idxf[:], acc[:], -1.0)
                        else:
                            # Sign(0)=+1: acc = (N-idx)-idx -> idx=(N-acc)/2
                            nc.vector.tensor_scalar(
                                idxf[:],
                                acc[:],
                                -0.5,
                                float(N) / 2.0,
                                mybir.AluOpType.mult,
                                mybir.AluOpType.add,
                            )
                    else:
                        gm = mpool.tile([128, 1], F32, tag="gm")
                        nc.vector.tensor_copy(gm[:], gmax)
                        junkf = spool.tile([128, N], F32, tag="junkf")
                        nc.vector.tensor_scalar(
                            junkf[:],
                            field[:],
                            gm[:],
                            None,
                            mybir.AluOpType.is_lt,
                            accum_out=idxf[:],
                        )
                    nc.vector.tensor_copy(idxstage[:, t : t + 1], idxf[:])
                g = gpool.tile([128, D], F32, tag="g")
                nc.gpsimd.indirect_dma_start(
                    out=g[:],
                    out_offset=None,
                    in_=emb_d[:],
                    in_offset=bass.IndirectOffsetOnAxis(
                        ap=idxstage[:, t : t + 1], axis=0
                    ),
                )
                nc.sync.dma_start(q_d[t * 128 : (t + 1) * 128, :], g[:])
            nc.sync.dma_start(i_d.rearrange("(t p) -> p t", p=128), idxstage[:])
    nc.compile()
    return nc


def _hilo16(a32):
    """Split fp32 array into (hi, lo) fp16 pair with hi+lo ~= a to ~2^-23."""
    hi = a32.astype(np.float16)
    lo = (a32 - hi.astype(np.float32)).astype(np.float16)
    return hi, lo


def _prep_inputs(x, embed):
    x = np.ascontiguousarray(np.asarray(x), dtype=np.float32)
    embed = np.ascontiguousarray(np.asarray(embed), dtype=np.float32)
    xf = x.reshape(B * T, D)
    nh = (-0.5 * np.sum(embed * embed, axis=1, dtype=np.float32)).astype(
        np.float32
    )
    nh_hi, nh_lo = _hilo16(nh)
    nhesq = np.stack([nh_hi, nh_lo]).reshape(2, N)
    nhesq32 = nh.reshape(1, N)
    eT = np.ascontiguousarray(embed.T)            # [256, 8192] f32
    e_hi, e_lo = _hilo16(eT)
    embedT = np.concatenate(
        [e_hi.reshape(2, 128, N), e_lo.reshape(2, 128, N)], axis=0
    )                                              # [4,128,N] f16
    in_maps = []
    for c in range(NCORES):
        xs = np.ascontiguousarray(xf[c * TPC : (c + 1) * TPC].T)  # [256,TPC]
        x_hi, x_lo = _hilo16(xs)
        xTc = np.concatenate(
            [x_hi.reshape(2, 128, TPC), x_lo.reshape(2, 128, TPC)], axis=0
        )                                          # [4,128,TPC] f16
        in_maps.append(
            {
                "xT": xTc,
                "embedT": embedT,
                "embed": embed,
                "nhesq": nhesq,
                "nhesq32": nhesq32,
            }
        )
    return in_maps


def _postprocess(results):
    quant = np.concatenate([r["quant"] for r in results], axis=0)
    ind = np.concatenate([r["ind"] for r in results], axis=0)
    return (
        quant.reshape(B, T, D).astype(np.float32),
        ind.reshape(B, T).astype(np.int32),
    )


def run(x, embed, count_mode=COUNT_MODE, score_mode=SCORE_MODE, **run_kwargs):
    in_maps = _prep_inputs(x, embed)
    nc = _build_nc(count_mode, score_mode=score_mode)
    res = run_bass_kernel_spmd(nc, in_maps, core_ids=list(range(NCORES)), **run_kwargs)
    return _postprocess(res.results), res


def kernel(x, embed):
    (quant, ind), _ = run(x, embed)
    return quant, ind


# revision 41
# speedup vs baseline: 6.6507x; 1.0143x over previous
"""Trainium2 Bass kernel for EuclideanCodebook (VQ) encode+decode.

Computes, for x:[8,1024,256] f32 and embed:[8192,256] f32:
    ind[b,t]   = argmin_n ||x[b,t]-embed[n]||^2
    quant[b,t] = embed[ind[b,t]]

Sharding: data-parallel over the 8192 tokens across 8 NeuronCores
(1024 tokens/core); the codebook is replicated.

Per-core device algorithm (token tiles of 128 on partitions):
  - scores s[t,n] = x[t]·e[n] - 0.5||e[n]||^2  (argmax_n s == argmin_n dist)
    computed on the PE: two K=128 matmuls for x·e plus one "bias matmul"
    (lhsT = e0 selector row) that broadcasts -0.5||e||^2 into PSUM.
  - running cummax of s along n via DVE tensor_tensor_scan (PSUM->SBUF);
    the last element is the global max per token.
  - index = count of positions where cummax < max (== first argmax), taken
    with a single fused pass (ScalarE Sign+accum, or DVE is_lt+accum).
  - decode = indirect-DMA gather of embed rows from HBM by index.
"""

import numpy as np

import concourse.bass as bass
import concourse.tile as tile
from concourse import bacc, mybir
from concourse.bass_utils import run_bass_kernel_spmd

B, T, D, N = 8, 1024, 256, 8192
NCORES = 8
TPC = (B * T) // NCORES          # tokens per core
NTILES = TPC // 128              # token tiles per core
NCHUNK = 2048                    # psum chunk width (4 banks)
NP = N // NCHUNK                 # chunks per token tile
F32 = mybir.dt.float32
F16 = mybir.dt.float16
NEG = -3.0e38

# The PE runs fp32 matmuls at 1/4 rate (2 half-speed passes). Instead we
# split each fp32 operand into a fp16 hi+lo pair (x = hx+lx exactly to
# ~2^-23 rel) and compute hx*he + lx*he + hx*le in full-rate fp16 matmuls;
# fp16 11-bit products are exact in the fp32 PSUM accumulate, and the
# dropped lx*le term is ~1e-6 — same accuracy class as the fp32 matmul.

# How the count-pass is computed:
#   "act_sign": ScalarE activation(Sign)+accum  (assumes Sign(0)=0 -> idx=-acc)
#   "act_sign1": same but Sign(0)=+1 convention -> idx=(N-acc)/2
#   "dve": DVE tensor_scalar(is_lt)+accum       (idx=acc)
COUNT_MODE = "act_sign"

# Scoring/argmax strategy:
#   "scan": bias folded into PSUM via selector matmul; DVE cummax scan;
#           index = count(cummax < max) fused on ScalarE (or DVE)
#   "add_scan": bias DMA-broadcast from DRAM and added by DVE tensor_tensor
#           (no bias matmuls), then chunked DVE cummax scan; same count pass
#   "ttr":  (BROKEN on this HW: InstTensorTensorReduce faults the device)
SCORE_MODE = "add_scan"


def _build_nc(count_mode=COUNT_MODE, reps=1, score_mode=SCORE_MODE):
    nc = bacc.Bacc(
        "TRN2",
        target_bir_lowering=False,
        debug=False,
        num_devices=NCORES,
    )
    # hi/lo fp16 pairs, pre-split on host: index 0,1 = hx chunks, 2,3 = lx
    xT_d = nc.dram_tensor("xT", [4, 128, TPC], F16, kind="ExternalInput")
    eT_d = nc.dram_tensor("embedT", [4, 128, N], F16, kind="ExternalInput")
    emb_d = nc.dram_tensor("embed", [N, D], F32, kind="ExternalInput")
    # rows 0/1 = hi/lo fp16 of -0.5*||e||^2
    nh_d = nc.dram_tensor("nhesq", [2, N], F16, kind="ExternalInput")
    # same bias as a single fp32 row (DMA-broadcast source for add_scan)
    nh32_d = nc.dram_tensor("nhesq32", [1, N], F32, kind="ExternalInput")
    q_d = nc.dram_tensor("quant", [TPC, D], F32, kind="ExternalOutput")
    i_d = nc.dram_tensor("ind", [TPC], mybir.dt.int32, kind="ExternalOutput")

    with tile.TileContext(nc) as tc:
        with (
            tc.tile_pool(name="const", bufs=1) as cpool,
            tc.tile_pool(name="psum", bufs=2, space="PSUM") as ppool,
            tc.tile_pool(name="scores", bufs=1) as spool,
            tc.tile_pool(name="small", bufs=2) as mpool,
            tc.tile_pool(name="gath", bufs=2) as gpool,
        ):
            # ---- resident inputs -------------------------------------------------
            # Small control tensors are issued FIRST: DMA completion sems are
            # FIFO per lane, so anything queued after the 8MB embT stream
            # would wait behind it.
            if score_mode != "add_scan":
                # bias tile: rows 0/1 = hi/lo of -0.5*||e||^2, rest zeroed
                # (selector matmul multiplies them by 0, but NaN garbage
                # would poison 0*x)
                biasr = cpool.tile([128, N], F16)
                nc.vector.memset(biasr[:], 0.0)
                nc.gpsimd.dma_start(biasr[0:2, :], nh_d[:])
                # row0+row1 selector: out[m,n] = rhs[0,n] + rhs[1,n]
                sel = cpool.tile([128, 128], F16)
                nc.vector.memset(sel[:], 0.0)
                nc.vector.memset(sel[0:2, :], 1.0)
            neginf = cpool.tile([128, 1], F32)
            nc.vector.memset(neginf[:], NEG)
            xT = cpool.tile([128, 4, TPC], F16)
            for c in range(4):
                nc.gpsimd.dma_start(xT[:, c, :], xT_d[c])
            # embT arrives column-slice-major, interleaved with the bias
            # broadcast: per column slice, hi chunks (consumed by the first
            # four matmul pairs) first, then the DMA-broadcast fp32 bias row
            # (consumed by the DVE add right after the chunk's matmuls),
            # then the lo chunks.
            if score_mode == "add_scan":
                nhb = cpool.tile([128, N], F32)
            embT = cpool.tile([128, 4, N], F16)
            for p in range(NP):
                sl = slice(p * NCHUNK, (p + 1) * NCHUNK)
                for c in (0, 1):
                    nc.sync.dma_start(embT[:, c, sl], eT_d[c, :, sl])
                if score_mode == "add_scan":
                    nc.sync.dma_start(
                        nhb[:, sl],
                        nh32_d[0:1, sl].to_broadcast([128, NCHUNK]),
                    )
                for c in (2, 3):
                    nc.sync.dma_start(embT[:, c, sl], eT_d[c, :, sl])
            idxstage = cpool.tile([128, NTILES], mybir.dt.int32)

            if score_mode == "ttr":
                nhb = cpool.tile([128, N], F32)
                for p in range(NP):
                    bps = ppool.tile([128, NCHUNK], F32, tag="ps")
                    for s in range(NCHUNK // 512):
                        sl = slice(p * NCHUNK + s * 512, p * NCHUNK + (s + 1) * 512)
                        nc.tensor.matmul(
                            bps[:, s * 512 : (s + 1) * 512],
                            lhsT=sel[:],
                            rhs=biasr[:, sl],
                            start=True,
                            stop=True,
                        )
                    nc.scalar.activation(
                        out=nhb[:, p * NCHUNK : (p + 1) * NCHUNK],
                        in_=bps[:],
                        func=mybir.ActivationFunctionType.Copy,
                        scale=1.0,
                    )

            for _rep in range(reps):
              for t in range(NTILES):
                s_t = spool.tile([128, N], F32, tag="scores")
                if score_mode == "add_scan":
                    cum = spool.tile([128, N], F32, tag="cum")
                if score_mode == "ttr":
                    bmax = mpool.tile([128, 8], F32, tag="bmax")
                    nc.vector.memset(bmax[:], NEG)
                for p in range(NP):
                    ps = ppool.tile([128, NCHUNK], F32, tag="ps")
                    nsl = [
                        slice((p * NCHUNK + s * 512), (p * NCHUNK + (s + 1) * 512))
                        for s in range(NCHUNK // 512)
                    ]
                    if score_mode == "scan":
                        for s, sl in enumerate(nsl):
                            nc.tensor.matmul(
                                ps[:, s * 512 : (s + 1) * 512],
                                lhsT=sel[:],
                                rhs=biasr[:, sl],
                                start=True,
                                stop=False,
                            )
                    # (hx,he), (lx,he), (hx,le) term chunks
                    pairs = [(0, 0), (1, 1), (2, 0), (3, 1), (0, 2), (1, 3)]
                    for pi, (xc, ec) in enumerate(pairs):
                        for s, sl in enumerate(nsl):
                            nc.tensor.matmul(
                                ps[:, s * 512 : (s + 1) * 512],
                                lhsT=xT[:, xc, t * 128 : (t + 1) * 128],
                                rhs=embT[:, ec, sl],
                                start=(score_mode != "scan" and pi == 0),
                                stop=(pi == len(pairs) - 1),
                            )
                    sl_out = slice(p * NCHUNK, (p + 1) * NCHUNK)
                    if score_mode == "scan":
                        # cummax chunk (chained through prev chunk's last col)
                        init = (
                            NEG if p == 0
                            else s_t[:, p * NCHUNK - 1 : p * NCHUNK]
                        )
                        nc.vector.tensor_tensor_scan(
                            out=s_t[:, sl_out],
                            data0=ps[:],
                            data1=neginf[:].to_broadcast([128, NCHUNK]),
                            initial=init,
                            op0=mybir.AluOpType.max,
                            op1=mybir.AluOpType.max,
                        )
                    elif score_mode == "add_scan":
                        nc.vector.tensor_tensor(
                            out=s_t[:, sl_out],
                            in0=ps[:],
                            in1=nhb[:, sl_out],
                            op=mybir.AluOpType.add,
                        )
                        # chunk cummax chained through prev chunk's last col;
                        # interleaved with the adds so PSUM slots free at a
                        # steady cadence and the PE never stalls into HAM
                        # re-throttle
                        init = (
                            NEG if p == 0
                            else cum[:, p * NCHUNK - 1 : p * NCHUNK]
                        )
                        nc.vector.tensor_tensor_scan(
                            out=cum[:, sl_out],
                            data0=s_t[:, sl_out],
                            data1=neginf[:].to_broadcast([128, NCHUNK]),
                            initial=init,
                            op0=mybir.AluOpType.max,
                            op1=mybir.AluOpType.max,
                        )
                    else:
                        # s = xe + (-0.5||e||^2); running chunk max in accum
                        nc.vector.tensor_tensor_reduce(
                            out=s_t[:, sl_out],
                            in0=ps[:],
                            in1=nhb[:, sl_out],
                            scale=1.0,
                            scalar=NEG,
                            op0=mybir.AluOpType.add,
                            op1=mybir.AluOpType.max,
                            accum_out=bmax[:, p : p + 1],
                        )
                if score_mode == "ttr":
                    gmax8 = mpool.tile([128, 8], F32, tag="gmax8")
                    nc.vector.max(gmax8[:], bmax[:])
                    idx8 = mpool.tile([128, 8], mybir.dt.uint32, tag="idx8")
                    nc.vector.max_index(idx8[:], gmax8[:], s_t[:])
                    nc.vector.tensor_copy(
                        idxstage[:, t : t + 1], idx8[:, 0:1]
                    )
                else:
                    if score_mode == "add_scan":
                        field = cum
                        junkb = spool.tile([128, N], F16, tag="bjunk")
                        junk_ap = junkb[:]
                    else:
                        field = s_t
                        junk_ap = None
                    idxf = mpool.tile([128, 1], F32, tag="idxf")
                    gmax = field[:, N - 1 : N]
                    acc = mpool.tile([128, 1], F32, tag="acc")
                    if count_mode in ("act_sign", "act_sign1"):
                        ngmax = mpool.tile([128, 1], F32, tag="ngmax")
                        nc.vector.tensor_scalar_mul(ngmax[:], gmax, -1.0)
                        if junk_ap is None:
                            junk = spool.tile(
                                [128, N], mybir.dt.bfloat16, tag="junk"
                            )
                            junk_ap = junk[:]
                        nc.scalar.activation(
                            out=junk_ap,
                            in_=field[:],
                            func=mybir.ActivationFunctionType.Sign,
                            bias=ngmax[:],
                            scale=1.0,
                            accum_out=acc[:],
                        )
                        if count_mode == "act_sign":
                            # cummax<gmax -> -1; ==gmax -> 0: acc = -idx
                            nc.vector.tensor_scalar_mul(idxf[:], acc[:], -1.0)
                        else:
                            # Sign(0)=+1: acc = (N-idx)-idx -> idx=(N-acc)/2
                            nc.vector.tensor_scalar(
                                idxf[:],
                                acc[:],
                                -0.5,
                                float(N) / 2.0,
                                mybir.AluOpType.mult,
                                mybir.AluOpType.add,
                            )
                    else:
                        gm = mpool.tile([128, 1], F32, tag="gm")
                        nc.vector.tensor_copy(gm[:], gmax)
                        junkf = spool.tile([128, N], F32, tag="junkf")
                        nc.vector.tensor_scalar(
                            junkf[:],
                            field[:],
                            gm[:],
                            None,
                            mybir.AluOpType.is_lt,
                            accum_out=idxf[:],
                        )
                    nc.vector.tensor_copy(idxstage[:, t : t + 1], idxf[:])
                g = gpool.tile([128, D], F32, tag="g")
                nc.gpsimd.indirect_dma_start(
                    out=g[:],
                    out_offset=None,
                    in_=emb_d[:],
                    in_offset=bass.IndirectOffsetOnAxis(
                        ap=idxstage[:, t : t + 1], axis=0
                    ),
                )
                nc.sync.dma_start(q_d[t * 128 : (t + 1) * 128, :], g[:])
            nc.sync.dma_start(i_d.rearrange("(t p) -> p t", p=128), idxstage[:])
    nc.compile()
    return nc


def _hilo16(a32):
    """Split fp32 array into (hi, lo) fp16 pair with hi+lo ~= a to ~2^-23."""
    hi = a32.astype(np.float16)
    lo = (a32 - hi.astype(np.float32)).astype(np.float16)
    return hi, lo


def _prep_inputs(x, embed):
    x = np.ascontiguousarray(np.asarray(x), dtype=np.float32)
    embed = np.ascontiguousarray(np.asarray(embed), dtype=np.float32)
    xf = x.reshape(B * T, D)
    nh = (-0.5 * np.sum(embed * embed, axis=1, dtype=np.float32)).astype(
        np.float32
    )
    nh_hi, nh_lo = _hilo16(nh)
    nhesq = np.stack([nh_hi, nh_lo]).reshape(2, N)
    nhesq32 = nh.reshape(1, N)
    eT = np.ascontiguousarray(embed.T)            # [256, 8192] f32
    e_hi, e_lo = _hilo16(eT)
    embedT = np.concatenate(
        [e_hi.reshape(2, 128, N), e_lo.reshape(2, 128, N)], axis=0
    )                                              # [4,128,N] f16
    in_maps = []
    for c in range(NCORES):
        xs = np.ascontiguousarray(xf[c * TPC : (c + 1) * TPC].T)  # [256,TPC]
        x_hi, x_lo = _hilo16(xs)
        xTc = np.concatenate(
            [x_hi.reshape(2, 128, TPC), x_lo.reshape(2, 128, TPC)], axis=0
        )                                          # [4,128,TPC] f16
        in_maps.append(
            {
                "xT": xTc,
                "embedT": embedT,
                "embed": embed,
                "nhesq": nhesq,
                "nhesq32": nhesq32,
            }
        )
    return in_maps


def _postprocess(results):
    quant = np.concatenate([r["quant"] for r in results], axis=0)
    ind = np.concatenate([r["ind"] for r in results], axis=0)
    return (
        quant.reshape(B, T, D).astype(np.float32),
        ind.reshape(B, T).astype(np.int32),
    )


def run(x, embed, count_mode=COUNT_MODE, score_mode=SCORE_MODE, **run_kwargs):
    in_maps = _prep_inputs(x, embed)
    nc = _build_nc(count_mode, score_mode=score_mode)
    res = run_bass_kernel_spmd(nc, in_maps, core_ids=list(range(NCORES)), **run_kwargs)
    return _postprocess(res.results), res


def kernel(x, embed):
    (quant, ind), _ = run(x, embed)
    return quant, ind
